# revision 1
# baseline (speedup 1.0000x reference)
"""CrissCrossAttention on 8 TRN2 NeuronCores.

Sharding: core = 2*b + hh  (b in 0..3 batches, hh in 0..1 head-halves).
Each core computes 4 heads' criss-cross attention for one batch element
plus its row-slice of the output projection; the host sums the two
head-half partials and adds the bias.

On-device dataflow (bf16 compute, f32 psum accumulation):
  xT   (DMA transpose)  ->  qT/kT [hd, t] + v in two layouts
  S^T  = kT' q          ->  exp (ACT, scale fused)  ->  denom (ones-matmul)
  attn@V (lhsT = V)     ->  TT-mul by recip(denom) during PSUM evacuation
  out-proj (lhsT = outT, rhs = w_out rows)  ->  natural-layout partial
"""

import numpy as np
import ml_dtypes

H = 8
C = 64
NP = 128
D = 512
HD = 64
B = 4
L = C * NP
HL = 4            # local heads per core
DHL = HL * HD     # 256 local head dims
SCALE = HD ** -0.5

_CACHE: dict = {}


def _build():
    import concourse.mybir as mybir
    import concourse.tile as tile
    from concourse import bacc

    dt = mybir.dt
    BF16 = dt.bfloat16
    F32 = dt.float32
    AFT = mybir.ActivationFunctionType

    nc = bacc.Bacc(
        "TRN2", target_bir_lowering=False, debug=False, enable_asserts=False
    )
    x = nc.dram_tensor("x", [L, D], BF16, kind="ExternalInput").ap()
    wq = nc.dram_tensor("wq", [D, DHL], BF16, kind="ExternalInput").ap()
    wk = nc.dram_tensor("wk", [D, DHL], BF16, kind="ExternalInput").ap()
    wv = nc.dram_tensor("wv", [D, DHL], BF16, kind="ExternalInput").ap()
    wo = nc.dram_tensor("wo", [DHL, D], BF16, kind="ExternalInput").ap()
    out = nc.dram_tensor("out", [L, D], F32, kind="ExternalOutput").ap()

    with tile.TileContext(nc) as tc, tc.tile_pool(name="persist", bufs=1) as pp:
        wq_s = pp.tile([128, 4 * DHL], BF16, tag="wq_s")
        wk_s = pp.tile([128, 4 * DHL], BF16, tag="wk_s")
        wv_s = pp.tile([128, 4 * DHL], BF16, tag="wv_s")
        for ki in range(4):
            ksl = slice(ki * DHL, (ki + 1) * DHL)
            rsl = slice(ki * 128, (ki + 1) * 128)
            nc.sync.dma_start(out=wq_s[:, ksl], in_=wq[rsl, :])
            nc.sync.dma_start(out=wk_s[:, ksl], in_=wk[rsl, :])
            nc.sync.dma_start(out=wv_s[:, ksl], in_=wv[rsl, :])
        wo_s = pp.tile([128, 2 * D], BF16, tag="wo_s")
        for hp in range(2):
            nc.sync.dma_start(
                out=wo_s[:, hp * D : (hp + 1) * D],
                in_=wo[hp * 128 : (hp + 1) * 128, :],
            )
        ones = pp.tile([128, 128], BF16, tag="ones")
        nc.vector.memset(ones[:], 1.0)

        qT = [pp.tile([128, L], BF16, tag=f"qT{i}", name=f"qT{i}") for i in range(2)]
        kT = [pp.tile([128, L], BF16, tag=f"kT{i}", name=f"kT{i}") for i in range(2)]
        # vA[p=n, c*DHL + h*HD + dh]  (temporal keys on partitions)
        vA = pp.tile([128, C * DHL], BF16, tag="vA")
        # vS[p=64*(nt%2)+c, (nt//2)*DHL + h*HD + dh] (spatial keys on partitions)
        vS = pp.tile([128, (NP // 2) * DHL], BF16, tag="vS")

        # ---------------- Phase 1: xT + QKV projections ----------------
        with (
            tc.tile_pool(name="xp", bufs=1) as xp,
            tc.tile_pool(name="psQ", bufs=2, space="PSUM") as psQp,
            tc.tile_pool(name="psV", bufs=2, space="PSUM") as psVp,
            tc.tile_pool(name="psW", bufs=4, space="PSUM") as psWp,
        ):
            xk = [
                xp.tile([128, L], BF16, tag=f"xk{i}", name=f"xk{i}")
                for i in range(4)
            ]
            for ki in range(4):
                nc.sync.dma_start(
                    out=xk[ki][:],
                    in_=x[:, ki * 128 : (ki + 1) * 128],
                    transpose=True,
                )

            # q/k transposed projections: psum [128, 512] chunks
            for tch in range(16):
                sl = slice(tch * 512, (tch + 1) * 512)
                for hp in range(2):
                    for wsb, dst in ((wq_s, qT[hp]), (wk_s, kT[hp])):
                        ps = psQp.tile([128, 512], F32, tag="psQ", name="psq")
                        for ki in range(4):
                            lo = ki * DHL + hp * 128
                            nc.tensor.matmul(
                                ps[:],
                                wsb[:, lo : lo + 128],
                                xk[ki][:, sl],
                                start=(ki == 0),
                                stop=(ki == 3),
                            )
                        nc.scalar.copy(out=dst[:, sl], in_=ps[:])

            # vA: natural v, contiguous t-tiles
            for tt in range(C):
                ps = psVp.tile([128, DHL], F32, tag="psV", name="psv")
                tsl = slice(tt * 128, (tt + 1) * 128)
                for ki in range(4):
                    nc.tensor.matmul(
                        ps[:],
                        xk[ki][:, tsl],
                        wv_s[:, ki * DHL : (ki + 1) * DHL],
                        start=(ki == 0),
                        stop=(ki == 3),
                    )
                nc.vector.tensor_copy(
                    out=vA[:, tt * DHL : (tt + 1) * DHL], in_=ps[:]
                )

            # vS: strided (channel-on-partition) v tiles, parity-packed.
            # Even/odd nt share one psum tile via col-groups -> concurrent MMs.
            for np2 in range(NP // 2):
                # separate psum tiles (= separate banks): interleaved start=True
                # chains in one bank would clear each other's has_written bits
                ps = [
                    psWp.tile([128, DHL], F32, tag="psW", name="psw"),
                    psWp.tile([128, DHL], F32, tag="psW", name="psw"),
                ]
                for ki in range(4):
                    for par in range(2):
                        nt = 2 * np2 + par
                        nc.tensor.matmul(
                            ps[par][64 * par : 64 * par + 64, :],
                            xk[ki][:, nt :: NP],
                            wv_s[:, ki * DHL : (ki + 1) * DHL],
                            start=(ki == 0),
                            stop=(ki == 3),
                            tile_position=(0, 64 * par),
                        )
                for par in range(2):
                    b = 64 * par
                    nc.vector.tensor_copy(
                        out=vS[b : b + 64, np2 * DHL : (np2 + 1) * DHL],
                        in_=ps[par][b : b + 64, :],
                    )

        # ---------------- Phase 2: criss-cross attention ----------------
        with tc.tile_pool(name="persist2", bufs=1) as pp2:
          # oT[p = 64*(h%2)+dh, c*128+n] per head-pair: out_s^T + out_t^T
          oT = [
              pp2.tile([128, L], BF16, tag=f"oT{i}", name=f"oT{i}")
              for i in range(2)
          ]
          with (
            tc.tile_pool(name="psS", bufs=2, space="PSUM") as psSp,
            tc.tile_pool(name="psD", bufs=3, space="PSUM") as psDp,
            tc.tile_pool(name="psO", bufs=3, space="PSUM") as psOp,
            tc.tile_pool(name="esP", bufs=4) as esP,
            tc.tile_pool(name="rcP", bufs=4) as rcP,
            tc.tile_pool(name="oSP", bufs=1) as oSP,
          ):
            oS = oSP.tile([128, L], BF16, tag="oS")
            for h in range(HL):
                hp = h // 2
                ho = 64 * (h % 2)
                hsl = slice(ho, ho + 64)

                # ---- temporal: attend across n within each channel c ----
                for cg in range(16):
                    psS = psSp.tile([128, 512], F32, tag="psS", name="pss")
                    for j in range(4):
                        c = cg * 4 + j
                        csl = slice(c * 128, (c + 1) * 128)
                        nc.tensor.matmul(
                            psS[:, j * 128 : (j + 1) * 128],
                            kT[hp][hsl, csl],
                            qT[hp][hsl, csl],
                            start=True,
                            stop=True,
                        )
                    es = esP.tile([128, 512], BF16, tag="es", name="es")
                    nc.scalar.activation(
                        out=es[:], in_=psS[:], func=AFT.Exp, scale=SCALE
                    )
                    psd = psDp.tile([128, 512], F32, tag="psD", name="psd")
                    nc.tensor.matmul(
                        psd[:], ones[:, 0:128], es[:], start=True, stop=True
                    )
                    rc = rcP.tile([128, 512], BF16, tag="rc", name="rc")
                    with nc.allow_low_precision(reason="softmax recip bf16"):
                        nc.vector.reciprocal(out=rc[hsl, :], in_=psd[hsl, :])
                    pso = psOp.tile([128, 512], F32, tag="psO", name="pso")
                    for j in range(4):
                        c = cg * 4 + j
                        vlo = c * DHL + h * HD
                        nc.tensor.matmul(
                            pso[hsl, j * 128 : (j + 1) * 128],
                            vA[:, vlo : vlo + HD],
                            es[:, j * 128 : (j + 1) * 128],
                            start=True,
                            stop=True,
                            tile_position=(0, ho),
                        )
                    nc.vector.tensor_mul(
                        out=oT[hp][hsl, cg * 512 : (cg + 1) * 512],
                        in0=pso[hsl, :],
                        in1=rc[hsl, :],
                    )

                # ---- spatial: attend across c at each patch position n ----
                # Parities interleaved: consecutive MMs hit disjoint PE
                # row-groups (rows 0-63 vs 64-127) and run concurrently.
                for ng in range(8):
                    psS = psSp.tile([128, 512], F32, tag="psS", name="pss")
                    for j in range(8):
                        for par in range(2):
                            kb = 64 * par
                            nt = par + 2 * (ng * 8 + j)
                            nc.tensor.matmul(
                                psS[kb : kb + 64, j * 64 : (j + 1) * 64],
                                kT[hp][hsl, nt::NP],
                                qT[hp][hsl, nt::NP],
                                start=True,
                                stop=True,
                                tile_position=(ho, kb),
                            )
                    es = esP.tile([128, 512], BF16, tag="es", name="es")
                    nc.scalar.activation(
                        out=es[:], in_=psS[:], func=AFT.Exp, scale=SCALE
                    )
                    psd = [None, None]
                    rc = [None, None]
                    for par in range(2):
                        kb = 64 * par
                        psd[par] = psDp.tile(
                            [128, 512], F32, tag="psD", name="psd"
                        )
                        nc.tensor.matmul(
                            psd[par][:], ones[kb : kb + 64, 0:128],
                            es[kb : kb + 64, :], start=True, stop=True,
                        )
                        rc[par] = rcP.tile([128, 512], BF16, tag="rc", name="rc")
                        with nc.allow_low_precision(reason="softmax recip bf16"):
                            nc.vector.reciprocal(
                                out=rc[par][hsl, :], in_=psd[par][hsl, :]
                            )
                    pso = [None, None]
                    for par in range(2):
                        pso[par] = psOp.tile(
                            [128, 512], F32, tag="psO", name="pso"
                        )
                    for j in range(8):
                        for par in range(2):
                            kb = 64 * par
                            nt = par + 2 * (ng * 8 + j)
                            vlo = (nt // 2) * DHL + h * HD
                            nc.tensor.matmul(
                                pso[par][hsl, j * 64 : (j + 1) * 64],
                                vS[kb : kb + 64, vlo : vlo + HD],
                                es[kb : kb + 64, j * 64 : (j + 1) * 64],
                                start=True,
                                stop=True,
                                tile_position=(kb, ho),
                            )
                    o3 = oS[hsl, :].rearrange("p (n q) -> p n q", q=64)
                    for par in range(2):
                        # oS[p=dh, n*64+cq]; units nt = par+2*(ng*8+j)
                        osel = o3[:, par + 16 * ng : par + 16 * ng + 15 : 2, :]
                        nc.vector.tensor_mul(
                            out=osel,
                            in0=pso[par][hsl, :].rearrange("p (j q) -> p j q", j=8),
                            in1=rc[par][hsl, :].rearrange("p (j q) -> p j q", j=8),
                        )

                # fold spatial into oT: oT[dh, c*128+n] += oS[dh, n*64+c]
                oTv = oT[hp][hsl, :].rearrange("p (c n) -> p c n", n=NP)
                oSv = oS[hsl, :].rearrange("p (n q) -> p q n", q=64)
                nc.vector.tensor_add(out=oTv, in0=oTv, in1=oSv)

          # ---------------- Phase 3: output projection ----------------
          with (
              tc.tile_pool(name="psF", bufs=4, space="PSUM") as psFp,
              tc.tile_pool(name="obP", bufs=4) as obP,
          ):
              for tt in range(C):
                  psf = psFp.tile([128, 512], F32, tag="psF", name="psf")
                  tsl = slice(tt * 128, (tt + 1) * 128)
                  for hp in range(2):
                      nc.tensor.matmul(
                          psf[:],
                          oT[hp][:, tsl],
                          wo_s[:, hp * D : (hp + 1) * D],
                          start=(hp == 0),
                          stop=(hp == 1),
                      )
                  ob = obP.tile([128, 512], F32, tag="ob", name="ob")
                  nc.scalar.copy(out=ob[:], in_=psf[:])
                  nc.sync.dma_start(out=out[tsl, :], in_=ob[:])

    nc.compile()
    return nc


def _get_nc():
    if "nc" not in _CACHE:
        _CACHE["nc"] = _build()
    return _CACHE["nc"]


def _in_maps(x, w_qkv, w_out):
    bf = ml_dtypes.bfloat16
    in_maps = []
    for core in range(8):
        b, hh = divmod(core, 2)
        lo = hh * DHL
        in_maps.append(
            {
                "x": np.ascontiguousarray(x[b]).astype(bf),
                "wq": np.ascontiguousarray(w_qkv[:, lo : lo + DHL]).astype(bf),
                "wk": np.ascontiguousarray(w_qkv[:, D + lo : D + lo + DHL]).astype(bf),
                "wv": np.ascontiguousarray(
                    w_qkv[:, 2 * D + lo : 2 * D + lo + DHL]
                ).astype(bf),
                "wo": np.ascontiguousarray(w_out[lo : lo + DHL, :]).astype(bf),
            }
        )
    return in_maps


def kernel(x, w_qkv, w_out, b_out, trace=False):
    from concourse import bass_utils

    nc = _get_nc()
    res = bass_utils.run_bass_kernel_spmd(
        nc, _in_maps(x, w_qkv, w_out), core_ids=list(range(8)), trace=trace
    )
    _CACHE["last_results"] = res
    out = np.empty((B, L, D), dtype=np.float32)
    for b in range(B):
        out[b] = res.results[2 * b]["out"] + res.results[2 * b + 1]["out"] + b_out
    return out

